# revision 11
# baseline (speedup 1.0000x reference)
"""Fused cross-attention Bass/Tile kernel for Trainium2, batch-sharded over 8 cores.

Per core (one batch element):
  Phase 1+2 (projection, per 512-row chunk of ctx and x):
    Q^T = Wq @ x^T + bq      [D, NQ]   (e on partitions, bf16 in SBUF)
    K^T = Wk @ ctx^T + bk    [D, NK]   (bf16)
    V   = ctx @ Wv^T         [NK, D]   (bf16, bv deferred to the output)
  Phase 3 (attention, per 512-query chunk), pure PE streaming:
    S^T = K^T.T-contraction: S^T[m, n] = sum_e K^T[e,m] Q^T[e,n]   (PE, bf16)
    E^T = exp(scale * S^T)   (ACT, PSUM->SBUF, bf16)
    O   += E^T.T @ V         (PE accumulation over m-tiles)
    eacc += E^T              (Pool engine, SBUF f32 accumulator)
    rs   = reduce(eacc.T)    (PE transpose + DVE free-dim reduce, per q-chunk)
    out = O / rs + bv        (DVE scalar_tensor_tensor)

All attention operands (K^T, Q^T, V, E^T) are stored bf16: the PE streams
1 column/cycle for bf16 and f32r alike, so this costs nothing on the PE,
but it halves SBUF footprint -- letting Q^T for ALL query chunks be
precomputed during phase 1+2.  Phase 3 then has no projection preamble at
all: no PSUM contention between Q-projection and scores (ps pool gets 4
banks of lookahead), and no per-chunk dependency stalls.  bf16 stationary
operands also enable the PE's automatic fast-weight-load path.

The S^T orientation means softmax normalization needs no P-transpose and the
PV matmul consumes E^T directly as the stationary operand.  Row sums are
accumulated on the (otherwise idle) Pool engine.
"""

import contextlib
import os
import sys

if "/opt/trn_rl_repo" not in sys.path:
    sys.path.insert(0, "/opt/trn_rl_repo")

# The PJRT neuron plugin consults its NEFF cache keyed on the XLA module
# fingerprint, which ignores the bass_exec custom-call backend_config where
# the actual kernel BIR lives.  Two different Bass kernels with identical
# tensor shapes/names therefore collide and a stale NEFF gets loaded
# (--no_cache in NEURON_CC_FLAGS does not reliably reach the lookup).  The
# only robust guard is to physically drop the cache before compiling.
import shutil


def _purge_neff_cache():
    shutil.rmtree("/root/.neuron-compile-cache", ignore_errors=True)

import ml_dtypes
import numpy as np

import concourse.bass as bass
import concourse.mybir as mybir
import concourse.tile as tile
from concourse.bass_utils import run_bass_kernel_spmd
from concourse.masks import make_identity

P = 128
N_CORES = 8
F32 = mybir.dt.float32
F32R = mybir.dt.float32r
BF16 = mybir.dt.bfloat16


def _split_drain_waits(nc):
    """Walrus CoreV3 codegen rejects instructions carrying more than one sync
    wait in several encodings (TPB_CTRL drains, S3_LW fused-weight matmuls).
    Move all waits of any multi-wait instruction onto preceding single-wait
    NOPs on the same engine — the engine executes them in order, so the
    semantics are identical."""
    for bb in nc.m.functions[0].blocks:
        new_insts = []
        for inst in bb.instructions:
            if (
                inst.sync_info
                and inst.sync_info.on_wait
                and len(inst.sync_info.on_wait) > 1
            ):
                waits = list(inst.sync_info.on_wait)
                for k, w in enumerate(waits[:-1]):
                    new_insts.append(
                        mybir.InstNoOp(
                            name=f"{inst.name}_wsplit{k}",
                            engine=inst.engine,
                            ins=[],
                            outs=[],
                            sync_info=mybir.SyncInfo(on_wait=[w], on_update=[]),
                        )
                    )
                inst.sync_info.on_wait = [waits[-1]]
            new_insts.append(inst)
        bb.instructions[:] = new_insts


def build_attention(NQ=4096, NK=4096, D=512, split_drains=True, repeat3=1,
                    nonce=0):
    """nonce>0 adds a dummy [1, nonce] input: the PJRT NEFF cache keys on the
    HLO fingerprint, which ignores the embedded BIR — distinct nonce values
    force distinct fingerprints so different kernel builds can never collide.
    """
    assert NQ % 512 == 0 and NK % 512 == 0 and D == 512
    DC = D // P          # 4 contraction chunks
    EC = D // P          # 4 output-feature chunks
    N_QC = NQ // 512     # q-chunks of 512 queries
    N_MC = NK // 512     # m-chunks of 512 keys
    N_MT = NK // P       # m-tiles of 128 keys
    SCALE = 1.0 / float(np.sqrt(D))

    nc = bass.Bass("TRN2", target_bir_lowering=False, debug=False,
                   num_devices=N_CORES)

    # x/context/weights arrive pre-transposed and bf16 from the host
    # (layout marshalling, same rounding the kernel applied on-chip before)
    x_d = nc.dram_tensor("x", [D, NQ], BF16, kind="ExternalInput").ap()
    ctx_d = nc.dram_tensor("context", [D, NK], BF16, kind="ExternalInput").ap()
    wq_d = nc.dram_tensor("Wq", [D, D], BF16, kind="ExternalInput").ap()
    bq_d = nc.dram_tensor("bq", [D], F32, kind="ExternalInput").ap()
    wk_d = nc.dram_tensor("Wk", [D, D], BF16, kind="ExternalInput").ap()
    bk_d = nc.dram_tensor("bk", [D], F32, kind="ExternalInput").ap()
    wv_d = nc.dram_tensor("Wv", [D, D], BF16, kind="ExternalInput").ap()
    bv_d = nc.dram_tensor("bv", [D], F32, kind="ExternalInput").ap()
    out_d = nc.dram_tensor("out", [NQ, D], F32, kind="ExternalOutput").ap()
    nonce_d = (nc.dram_tensor("nonce", [1, nonce], F32, kind="ExternalInput")
               .ap() if nonce else None)

    with tile.TileContext(nc) as tc:
        with (
            tc.tile_pool(name="consts", bufs=1) as consts,
            tc.tile_pool(name="persist", bufs=1) as persist,
        ):
            if nonce_d is not None:
                nonce_sb = consts.tile([1, nonce], F32)
                nc.sync.dma_start(out=nonce_sb, in_=nonce_d)
            ident = consts.tile([P, P], F32)
            make_identity(nc, ident)
            ident_bf = consts.tile([P, P], BF16)
            nc.scalar.copy(ident_bf, ident)
            bq_sb = consts.tile([P, EC], F32)
            nc.gpsimd.dma_start(out=bq_sb, in_=bq_d.rearrange("(c p) -> p c", p=P))
            bk_sb = consts.tile([P, EC], F32)
            nc.gpsimd.dma_start(out=bk_sb, in_=bk_d.rearrange("(c p) -> p c", p=P))
            bv_bcast = consts.tile([P, D], F32)
            nc.gpsimd.dma_start(
                out=bv_bcast,
                in_=bass.AP(tensor=bv_d.tensor, offset=bv_d.offset,
                            ap=[[0, P], *bv_d.ap]),
            )

            KT_sb = persist.tile([P, EC, NK], BF16)     # K^T: [e-part, ec, m]
            QT_sb = persist.tile([P, EC, NQ], BF16)     # Q^T: [e-part, ec, n]
            V_sb = persist.tile([P, N_MT, D], BF16)     # V:   [m-part, mt, e]

            rep = (tc.For_i(0, repeat3, 1) if repeat3 > 1
                   else contextlib.nullcontext())
            with rep:
             with (
                 tc.tile_pool(name="wkv", bufs=1) as wkv,
                 tc.tile_pool(name="pk2", bufs=2, space="PSUM") as pk2,
                 tc.tile_pool(name="pv2", bufs=2, space="PSUM") as pv2,
                 tc.tile_pool(name="pq2", bufs=2, space="PSUM") as pq2,
             ):
                 WkT_sb = wkv.tile([P, DC, D], BF16)
                 WvT_sb = wkv.tile([P, DC, D], BF16)
                 WqT_sb = wkv.tile([P, DC, D], BF16)
                 ctxT_sb = wkv.tile([P, DC, NK], BF16)
                 xT_sb = wkv.tile([P, DC, NQ], BF16)

                 # ---- Phase 1+2: all operands arrive pre-transposed, so the
                 # phase is pure projection matmuls.  ctx^T/Wk/Wv ride the SP
                 # DMA queue, x^T/Wq the Activation queue; per-chunk DMAs
                 # into subviews keep the PE fed from the first chunk on.
                 nc.sync.dma_start(
                     out=WkT_sb, in_=wk_d.rearrange("(c p) e -> p c e", p=P))
                 nc.scalar.dma_start(
                     out=WqT_sb, in_=wq_d.rearrange("(c p) e -> p c e", p=P))
                 nc.sync.dma_start(
                     out=WvT_sb, in_=wv_d.rearrange("(c p) e -> p c e", p=P))

                 def load_ctx(mc):
                     nc.sync.dma_start(
                         out=ctxT_sb[:, 0:DC, mc * 512:(mc + 1) * 512],
                         in_=ctx_d[:, mc * 512:(mc + 1) * 512]
                         .rearrange("(c p) m -> p c m", p=P))

                 def load_x(mc):
                     nc.scalar.dma_start(
                         out=xT_sb[:, 0:DC, mc * 512:(mc + 1) * 512],
                         in_=x_d[:, mc * 512:(mc + 1) * 512]
                         .rearrange("(c p) n -> p c n", p=P))

                 def kv_chunk(mc):
                     # K^T[:, mc chunk] = Wk @ ctx^T  (+bk on evacuation)
                     for ec in range(EC):
                         p_k = pk2.tile([P, 512], F32, tag="pk")
                         for dc in range(DC):
                             nc.tensor.matmul(
                                 p_k,
                                 WkT_sb[:, dc, ec * P:(ec + 1) * P],
                                 ctxT_sb[:, dc, mc * 512:(mc + 1) * 512],
                                 start=(dc == 0), stop=(dc == DC - 1))
                         nc.scalar.activation(
                             KT_sb[:, ec, mc * 512:(mc + 1) * 512], p_k,
                             mybir.ActivationFunctionType.Identity,
                             bias=bk_sb[:, ec:ec + 1], scale=1.0)
                     # V rows (no bias)
                     for jt in range(4):
                         p_v = pv2.tile([P, D], F32, tag="pv")
                         for dc in range(DC):
                             nc.tensor.matmul(
                                 p_v,
                                 ctxT_sb[:, dc, mc * 512 + jt * P:
                                         mc * 512 + (jt + 1) * P],
                                 WvT_sb[:, dc, :],
                                 start=(dc == 0), stop=(dc == DC - 1))
                         nc.vector.tensor_copy(V_sb[:, mc * 4 + jt, :], p_v)

                 def q_chunk(qc):
                     # Q^T[:, qc chunk] = Wq @ x^T  (+bq on evacuation)
                     for ec in range(EC):
                         p_q = pq2.tile([P, 512], F32, tag="pq")
                         for dc in range(DC):
                             nc.tensor.matmul(
                                 p_q,
                                 WqT_sb[:, dc, ec * P:(ec + 1) * P],
                                 xT_sb[:, dc, qc * 512:(qc + 1) * 512],
                                 start=(dc == 0), stop=(dc == DC - 1))
                         nc.scalar.activation(
                             QT_sb[:, ec, qc * 512:(qc + 1) * 512], p_q,
                             mybir.ActivationFunctionType.Identity,
                             bias=bq_sb[:, ec:ec + 1], scale=1.0)

                 for mc in range(N_MC):
                     load_ctx(mc)
                     load_x(mc)
                     kv_chunk(mc)
                     q_chunk(mc)

             # ---- Phase 3: attention, per 512-query chunk ----
             with (
                 tc.tile_pool(name="p3e", bufs=4) as p3e,
                 tc.tile_pool(name="p3o", bufs=4) as p3o,
                 tc.tile_pool(name="p3r", bufs=4) as p3r,
                 tc.tile_pool(name="pacc", bufs=3) as pacc,
                 tc.tile_pool(name="ps", bufs=3, space="PSUM") as ps,
                 tc.tile_pool(name="pst", bufs=1, space="PSUM") as pst,
                 tc.tile_pool(name="po", bufs=4, space="PSUM") as po,
             ):
                 for qc in range(N_QC):
                     p_o = [po.tile([P, D], F32, tag="po", name=f"po{i}")
                            for i in range(4)]
                     eacc = pacc.tile([P, 512], F32, tag="eacc")

                     for mt in range(N_MT):
                         p_s = ps.tile([P, 512], F32, tag="ps")
                         for ec in range(EC):
                             nc.tensor.matmul(
                                 p_s,
                                 KT_sb[:, ec, mt * P:(mt + 1) * P],
                                 QT_sb[:, ec, qc * 512:(qc + 1) * 512],
                                 start=(ec == 0), stop=(ec == EC - 1))
                         ET = p3e.tile([P, 512], BF16, tag="ET")
                         nc.scalar.activation(
                             ET, p_s, mybir.ActivationFunctionType.Exp,
                             bias=0.0, scale=SCALE)
                         if mt == 0:
                             nc.gpsimd.tensor_copy(eacc, ET)
                         elif mt == N_MT - 1:
                             # last add on the idle DVE: halves the lag the
                             # PE's accT transposes wait on at the chunk end
                             nc.vector.tensor_add(eacc, eacc, ET)
                         else:
                             nc.gpsimd.tensor_add(eacc, eacc, ET)
                         for nt in range(4):
                             nc.tensor.matmul(
                                 p_o[nt], ET[:, nt * P:(nt + 1) * P],
                                 V_sb[:, mt, :],
                                 start=(mt == 0), stop=(mt == N_MT - 1))

                     # rowsum over m: transpose eacc on PE, free-dim reduce
                     # on DVE -> rs_T[p, c] = sum_m E[m, 128c+p].  High
                     # priority: the drain chain frees the po banks the next
                     # chunk's PV matmuls accumulate into.
                     accT = pst.tile([P, 4, P], F32, tag="accT")
                     for c in range(4):
                         nc.tensor.transpose(
                             accT[:, c, :], eacc[:, c * P:(c + 1) * P],
                             ident)
                     with tc.high_priority(offset=360):
                         rs_T = p3r.tile([P, 4], F32, tag="rsT")
                         nc.vector.tensor_reduce(
                             rs_T, accT, axis=mybir.AxisListType.X,
                             op=mybir.AluOpType.add)
                         rinv = p3r.tile([P, 4], F32, tag="rinv")
                         nc.vector.reciprocal(rinv, rs_T)
                         for nt in range(4):
                             o_sb = p3o.tile([P, D], F32, tag="osb")
                             nc.vector.scalar_tensor_tensor(
                                 out=o_sb, in0=p_o[nt],
                                 scalar=rinv[:, nt:nt + 1],
                                 in1=bv_bcast,
                                 op0=mybir.AluOpType.mult,
                                 op1=mybir.AluOpType.add)
                             nc.sync.dma_start(
                                 out=out_d[qc * 512 + nt * P:
                                           qc * 512 + (nt + 1) * P, :],
                                 in_=o_sb)

    if split_drains:
        _split_drain_waits(nc)
    return nc


_NC_CACHE = {}


def _get_nc(NQ, NK, D):
    key = (NQ, NK, D)
    if key not in _NC_CACHE:
        _NC_CACHE[key] = build_attention(NQ, NK, D)
    return _NC_CACHE[key]


def kernel(x, context, Wq, bq, Wk, bk, Wv, bv):
    x = np.asarray(x, dtype=np.float32)
    context = np.asarray(context, dtype=np.float32)
    Wq = np.asarray(Wq, dtype=np.float32)
    bq = np.asarray(bq, dtype=np.float32)
    Wk = np.asarray(Wk, dtype=np.float32)
    bk = np.asarray(bk, dtype=np.float32)
    Wv = np.asarray(Wv, dtype=np.float32)
    bv = np.asarray(bv, dtype=np.float32)

    B, NQ, D = x.shape
    NK = context.shape[1]
    assert B == N_CORES, f"expected batch {N_CORES}, got {B}"

    nc = _get_nc(NQ, NK, D)
    _purge_neff_cache()
    bf = ml_dtypes.bfloat16
    WqT, WkT, WvT = Wq.T.astype(bf), Wk.T.astype(bf), Wv.T.astype(bf)
    in_maps = [
        {
            "x": x[b].T.astype(bf),
            "context": context[b].T.astype(bf),
            "Wq": WqT, "bq": bq, "Wk": WkT, "bk": bk,
            "Wv": WvT, "bv": bv,
        }
        for b in range(B)
    ]
    # The axon-tunneled devices intermittently come up poisoned from a prior
    # session (NRT_EXEC_UNIT_UNRECOVERABLE on the first execution).  The
    # worker restarts after the failure, so resetting the jax backend and
    # retrying recovers.
    import time as _time
    last_err = None
    for attempt in range(3):
        try:
            res = run_bass_kernel_spmd(nc, in_maps, list(range(N_CORES)))
            return np.stack([res.results[b]["out"] for b in range(B)])
        except Exception as e:  # noqa: BLE001 - device-level flake, retried
            last_err = e
            import jax
            try:
                jax.clear_backends()
            except Exception:
                pass
            _time.sleep(15)
            _purge_neff_cache()
    raise last_err


# revision 13
# speedup vs baseline: 1.0300x; 1.0300x over previous
"""Fused cross-attention Bass/Tile kernel for Trainium2, batch-sharded over 8 cores.

Per core (one batch element):
  Phase 1+2 (projection, per 512-row chunk of ctx and x):
    Q^T = Wq @ x^T + bq      [D, NQ]   (e on partitions, bf16 in SBUF)
    K^T = Wk @ ctx^T + bk    [D, NK]   (bf16)
    V   = ctx @ Wv^T         [NK, D]   (bf16, bv deferred to the output)
  Phase 3 (attention, per 512-query chunk), pure PE streaming:
    S^T = K^T.T-contraction: S^T[m, n] = sum_e K^T[e,m] Q^T[e,n]   (PE, bf16)
    E^T = exp(scale * S^T)   (ACT, PSUM->SBUF, bf16)
    O   += E^T.T @ V         (PE accumulation over m-tiles)
    eacc += E^T              (Pool engine, SBUF f32 accumulator)
    rs   = reduce(eacc.T)    (PE transpose + DVE free-dim reduce, per q-chunk)
    out = O / rs + bv        (DVE scalar_tensor_tensor)

All attention operands (K^T, Q^T, V, E^T) are stored bf16: the PE streams
1 column/cycle for bf16 and f32r alike, so this costs nothing on the PE,
but it halves SBUF footprint -- letting Q^T for ALL query chunks be
precomputed during phase 1+2.  Phase 3 then has no projection preamble at
all: no PSUM contention between Q-projection and scores (ps pool gets 4
banks of lookahead), and no per-chunk dependency stalls.  bf16 stationary
operands also enable the PE's automatic fast-weight-load path.

The S^T orientation means softmax normalization needs no P-transpose and the
PV matmul consumes E^T directly as the stationary operand.  Row sums are
accumulated on the (otherwise idle) Pool engine.
"""

import contextlib
import os
import sys

if "/opt/trn_rl_repo" not in sys.path:
    sys.path.insert(0, "/opt/trn_rl_repo")

# The PJRT neuron plugin consults its NEFF cache keyed on the XLA module
# fingerprint, which ignores the bass_exec custom-call backend_config where
# the actual kernel BIR lives.  Two different Bass kernels with identical
# tensor shapes/names therefore collide and a stale NEFF gets loaded
# (--no_cache in NEURON_CC_FLAGS does not reliably reach the lookup).  The
# only robust guard is to physically drop the cache before compiling.
import shutil


def _purge_neff_cache():
    shutil.rmtree("/root/.neuron-compile-cache", ignore_errors=True)

import ml_dtypes
import numpy as np

import concourse.bass as bass
import concourse.mybir as mybir
import concourse.tile as tile
from concourse.bass_utils import run_bass_kernel_spmd
from concourse.masks import make_identity

P = 128
N_CORES = 8
F32 = mybir.dt.float32
F32R = mybir.dt.float32r
BF16 = mybir.dt.bfloat16


def _split_drain_waits(nc):
    """Walrus CoreV3 codegen rejects instructions carrying more than one sync
    wait in several encodings (TPB_CTRL drains, S3_LW fused-weight matmuls).
    Move all waits of any multi-wait instruction onto preceding single-wait
    NOPs on the same engine — the engine executes them in order, so the
    semantics are identical."""
    for bb in nc.m.functions[0].blocks:
        new_insts = []
        for inst in bb.instructions:
            if (
                inst.sync_info
                and inst.sync_info.on_wait
                and len(inst.sync_info.on_wait) > 1
            ):
                waits = list(inst.sync_info.on_wait)
                for k, w in enumerate(waits[:-1]):
                    new_insts.append(
                        mybir.InstNoOp(
                            name=f"{inst.name}_wsplit{k}",
                            engine=inst.engine,
                            ins=[],
                            outs=[],
                            sync_info=mybir.SyncInfo(on_wait=[w], on_update=[]),
                        )
                    )
                inst.sync_info.on_wait = [waits[-1]]
            new_insts.append(inst)
        bb.instructions[:] = new_insts


def build_attention(NQ=4096, NK=4096, D=512, split_drains=True, repeat3=1,
                    nonce=0):
    """nonce>0 adds a dummy [1, nonce] input: the PJRT NEFF cache keys on the
    HLO fingerprint, which ignores the embedded BIR — distinct nonce values
    force distinct fingerprints so different kernel builds can never collide.
    """
    assert NQ % 512 == 0 and NK % 512 == 0 and D == 512
    DC = D // P          # 4 contraction chunks
    EC = D // P          # 4 output-feature chunks
    N_QC = NQ // 512     # q-chunks of 512 queries
    N_MC = NK // 512     # m-chunks of 512 keys
    N_MT = NK // P       # m-tiles of 128 keys
    SCALE = 1.0 / float(np.sqrt(D))

    nc = bass.Bass("TRN2", target_bir_lowering=False, debug=False,
                   num_devices=N_CORES)

    # x/context/weights arrive pre-transposed and bf16 from the host
    # (layout marshalling, same rounding the kernel applied on-chip before)
    x_d = nc.dram_tensor("x", [D, NQ], BF16, kind="ExternalInput").ap()
    ctx_d = nc.dram_tensor("context", [D, NK], BF16, kind="ExternalInput").ap()
    wq_d = nc.dram_tensor("Wq", [D, D], BF16, kind="ExternalInput").ap()
    bq_d = nc.dram_tensor("bq", [D], F32, kind="ExternalInput").ap()
    wk_d = nc.dram_tensor("Wk", [D, D], BF16, kind="ExternalInput").ap()
    bk_d = nc.dram_tensor("bk", [D], F32, kind="ExternalInput").ap()
    wv_d = nc.dram_tensor("Wv", [D, D], BF16, kind="ExternalInput").ap()
    bv_d = nc.dram_tensor("bv", [D], F32, kind="ExternalInput").ap()
    out_d = nc.dram_tensor("out", [NQ, D], F32, kind="ExternalOutput").ap()
    nonce_d = (nc.dram_tensor("nonce", [1, nonce], F32, kind="ExternalInput")
               .ap() if nonce else None)

    with tile.TileContext(nc) as tc:
        with (
            tc.tile_pool(name="consts", bufs=1) as consts,
            tc.tile_pool(name="persist", bufs=1) as persist,
        ):
            if nonce_d is not None:
                nonce_sb = consts.tile([1, nonce], F32)
                nc.sync.dma_start(out=nonce_sb, in_=nonce_d)
            ident = consts.tile([P, P], F32)
            make_identity(nc, ident)
            ident_bf = consts.tile([P, P], BF16)
            nc.scalar.copy(ident_bf, ident)
            bq_sb = consts.tile([P, EC], F32)
            nc.gpsimd.dma_start(out=bq_sb, in_=bq_d.rearrange("(c p) -> p c", p=P))
            bk_sb = consts.tile([P, EC], F32)
            nc.gpsimd.dma_start(out=bk_sb, in_=bk_d.rearrange("(c p) -> p c", p=P))
            bv_bcast = consts.tile([P, D], F32)
            nc.gpsimd.dma_start(
                out=bv_bcast,
                in_=bass.AP(tensor=bv_d.tensor, offset=bv_d.offset,
                            ap=[[0, P], *bv_d.ap]),
            )

            KT_sb = persist.tile([P, EC, NK], BF16)     # K^T: [e-part, ec, m]
            QT_sb = persist.tile([P, EC, NQ], BF16)     # Q^T: [e-part, ec, n]
            V_sb = persist.tile([P, N_MT, D], BF16)     # V:   [m-part, mt, e]

            rep = (tc.For_i(0, repeat3, 1) if repeat3 > 1
                   else contextlib.nullcontext())
            with rep:
             with (
                 tc.tile_pool(name="wkv", bufs=1) as wkv,
                 tc.tile_pool(name="pk2", bufs=2, space="PSUM") as pk2,
                 tc.tile_pool(name="pv2", bufs=2, space="PSUM") as pv2,
                 tc.tile_pool(name="pq2", bufs=2, space="PSUM") as pq2,
             ):
                 WkT_sb = wkv.tile([P, DC, D], BF16)
                 WvT_sb = wkv.tile([P, DC, D], BF16)
                 WqT_sb = wkv.tile([P, DC, D], BF16)
                 ctxT_sb = wkv.tile([P, DC, NK], BF16)
                 xT_sb = wkv.tile([P, DC, NQ], BF16)

                 # ---- Phase 1+2: all operands arrive pre-transposed, so the
                 # phase is pure projection matmuls.  ctx^T/Wk/Wv ride the SP
                 # DMA queue, x^T/Wq the Activation queue; per-chunk DMAs
                 # into subviews keep the PE fed from the first chunk on.
                 nc.sync.dma_start(
                     out=WkT_sb, in_=wk_d.rearrange("(c p) e -> p c e", p=P))
                 nc.scalar.dma_start(
                     out=WqT_sb, in_=wq_d.rearrange("(c p) e -> p c e", p=P))

                 def load_ctx(mc):
                     nc.sync.dma_start(
                         out=ctxT_sb[:, 0:DC, mc * 512:(mc + 1) * 512],
                         in_=ctx_d[:, mc * 512:(mc + 1) * 512]
                         .rearrange("(c p) m -> p c m", p=P))

                 def load_x(mc):
                     nc.scalar.dma_start(
                         out=xT_sb[:, 0:DC, mc * 512:(mc + 1) * 512],
                         in_=x_d[:, mc * 512:(mc + 1) * 512]
                         .rearrange("(c p) n -> p c n", p=P))

                 def kv_chunk(mc):
                     # K^T[:, mc chunk] = Wk @ ctx^T  (+bk on evacuation)
                     for ec in range(EC):
                         p_k = pk2.tile([P, 512], F32, tag="pk")
                         for dc in range(DC):
                             nc.tensor.matmul(
                                 p_k,
                                 WkT_sb[:, dc, ec * P:(ec + 1) * P],
                                 ctxT_sb[:, dc, mc * 512:(mc + 1) * 512],
                                 start=(dc == 0), stop=(dc == DC - 1))
                         nc.scalar.activation(
                             KT_sb[:, ec, mc * 512:(mc + 1) * 512], p_k,
                             mybir.ActivationFunctionType.Identity,
                             bias=bk_sb[:, ec:ec + 1], scale=1.0)
                     # V rows (no bias)
                     for jt in range(4):
                         p_v = pv2.tile([P, D], F32, tag="pv")
                         for dc in range(DC):
                             nc.tensor.matmul(
                                 p_v,
                                 ctxT_sb[:, dc, mc * 512 + jt * P:
                                         mc * 512 + (jt + 1) * P],
                                 WvT_sb[:, dc, :],
                                 start=(dc == 0), stop=(dc == DC - 1))
                         nc.vector.tensor_add(
                             V_sb[:, mc * 4 + jt, :], p_v, bv_bcast)

                 def q_chunk(qc):
                     # Q^T[:, qc chunk] = Wq @ x^T  (+bq on evacuation)
                     for ec in range(EC):
                         p_q = pq2.tile([P, 512], F32, tag="pq")
                         for dc in range(DC):
                             nc.tensor.matmul(
                                 p_q,
                                 WqT_sb[:, dc, ec * P:(ec + 1) * P],
                                 xT_sb[:, dc, qc * 512:(qc + 1) * 512],
                                 start=(dc == 0), stop=(dc == DC - 1))
                         nc.scalar.activation(
                             QT_sb[:, ec, qc * 512:(qc + 1) * 512], p_q,
                             mybir.ActivationFunctionType.Identity,
                             bias=bq_sb[:, ec:ec + 1], scale=1.0)

                 load_ctx(0)
                 load_x(0)
                 nc.sync.dma_start(
                     out=WvT_sb, in_=wv_d.rearrange("(c p) e -> p c e", p=P))
                 kv_chunk(0)
                 q_chunk(0)
                 for mc in range(1, N_MC):
                     load_ctx(mc)
                     load_x(mc)
                     kv_chunk(mc)
                     q_chunk(mc)

             # ---- Phase 3: attention, per 512-query chunk ----
             with (
                 tc.tile_pool(name="p3e", bufs=4) as p3e,
                 tc.tile_pool(name="p3o", bufs=4) as p3o,
                 tc.tile_pool(name="p3r", bufs=4) as p3r,
                 tc.tile_pool(name="pacc", bufs=3) as pacc,
                 tc.tile_pool(name="ps", bufs=3, space="PSUM") as ps,
                 tc.tile_pool(name="pst", bufs=1, space="PSUM") as pst,
                 tc.tile_pool(name="po", bufs=4, space="PSUM") as po,
             ):
                 for qc in range(N_QC):
                     p_o = [po.tile([P, D], F32, tag="po", name=f"po{i}")
                            for i in range(4)]
                     eacc = pacc.tile([P, 512], F32, tag="eacc")

                     for mt in range(N_MT):
                         p_s = ps.tile([P, 512], F32, tag="ps")
                         for ec in range(EC):
                             nc.tensor.matmul(
                                 p_s,
                                 KT_sb[:, ec, mt * P:(mt + 1) * P],
                                 QT_sb[:, ec, qc * 512:(qc + 1) * 512],
                                 start=(ec == 0), stop=(ec == EC - 1))
                         ET = p3e.tile([P, 512], BF16, tag="ET")
                         nc.scalar.activation(
                             ET, p_s, mybir.ActivationFunctionType.Exp,
                             bias=0.0, scale=SCALE)
                         if mt == 0:
                             nc.gpsimd.tensor_copy(eacc, ET)
                         elif mt == N_MT - 1:
                             # last add on the idle DVE: halves the lag the
                             # PE's accT transposes wait on at the chunk end
                             nc.vector.tensor_add(eacc, eacc, ET)
                         else:
                             nc.gpsimd.tensor_add(eacc, eacc, ET)
                         for nt in range(4):
                             nc.tensor.matmul(
                                 p_o[nt], ET[:, nt * P:(nt + 1) * P],
                                 V_sb[:, mt, :],
                                 start=(mt == 0), stop=(mt == N_MT - 1))

                     # rowsum over m: transpose eacc on PE, free-dim reduce
                     # on DVE -> rs_T[p, c] = sum_m E[m, 128c+p].  High
                     # priority: the drain chain frees the po banks the next
                     # chunk's PV matmuls accumulate into.
                     accT = pst.tile([P, 4, P], F32, tag="accT")
                     for c in range(4):
                         nc.tensor.transpose(
                             accT[:, c, :], eacc[:, c * P:(c + 1) * P],
                             ident)
                     with tc.high_priority(offset=360):
                         rs_T = p3r.tile([P, 4], F32, tag="rsT")
                         nc.vector.tensor_reduce(
                             rs_T, accT, axis=mybir.AxisListType.X,
                             op=mybir.AluOpType.add)
                         rinv = p3r.tile([P, 4], F32, tag="rinv")
                         nc.vector.reciprocal(rinv, rs_T)
                         for nt in range(4):
                             o_sb = p3o.tile([P, D], F32, tag="osb")
                             nc.vector.tensor_scalar_mul(
                                 o_sb, p_o[nt], rinv[:, nt:nt + 1])
                             (nc.sync if nt % 2 == 0 else nc.scalar).dma_start(
                                 out=out_d[qc * 512 + nt * P:
                                           qc * 512 + (nt + 1) * P, :],
                                 in_=o_sb)

    if split_drains:
        _split_drain_waits(nc)
    return nc


_NC_CACHE = {}


def _get_nc(NQ, NK, D):
    key = (NQ, NK, D)
    if key not in _NC_CACHE:
        _NC_CACHE[key] = build_attention(NQ, NK, D)
    return _NC_CACHE[key]


def kernel(x, context, Wq, bq, Wk, bk, Wv, bv):
    x = np.asarray(x, dtype=np.float32)
    context = np.asarray(context, dtype=np.float32)
    Wq = np.asarray(Wq, dtype=np.float32)
    bq = np.asarray(bq, dtype=np.float32)
    Wk = np.asarray(Wk, dtype=np.float32)
    bk = np.asarray(bk, dtype=np.float32)
    Wv = np.asarray(Wv, dtype=np.float32)
    bv = np.asarray(bv, dtype=np.float32)

    B, NQ, D = x.shape
    NK = context.shape[1]
    assert B == N_CORES, f"expected batch {N_CORES}, got {B}"

    nc = _get_nc(NQ, NK, D)
    _purge_neff_cache()
    bf = ml_dtypes.bfloat16
    WqT, WkT, WvT = Wq.T.astype(bf), Wk.T.astype(bf), Wv.T.astype(bf)
    in_maps = [
        {
            "x": x[b].T.astype(bf),
            "context": context[b].T.astype(bf),
            "Wq": WqT, "bq": bq, "Wk": WkT, "bk": bk,
            "Wv": WvT, "bv": bv,
        }
        for b in range(B)
    ]
    # The axon-tunneled devices intermittently come up poisoned from a prior
    # session (NRT_EXEC_UNIT_UNRECOVERABLE on the first execution).  The
    # worker restarts after the failure, so resetting the jax backend and
    # retrying recovers.
    import time as _time
    last_err = None
    for attempt in range(3):
        try:
            res = run_bass_kernel_spmd(nc, in_maps, list(range(N_CORES)))
            return np.stack([res.results[b]["out"] for b in range(B)])
        except Exception as e:  # noqa: BLE001 - device-level flake, retried
            last_err = e
            import jax
            try:
                jax.clear_backends()
            except Exception:
                pass
            _time.sleep(15)
            _purge_neff_cache()
    raise last_err
